# revision 10
# baseline (speedup 1.0000x reference)
"""Trainium2 Bass kernel for nn_FCOSDMLNegHead2 (vq_codebook).

Sharding: data-parallel over batch B=8 across 8 NeuronCores (1 image/core).
Per core: conv1 3x3 (256->256, BN folded on host) + conv2 3x3 (256->128) as
9-tap matmuls in bf16, per-pixel L2 norm, dot with 320 host-precomputed
prototypes, elementwise epilogue producing the 5 outputs.
"""

import os
import sys

import numpy as np
import ml_dtypes

for _p in ("/opt/trn_rl_repo", "/root/.axon_site/_ro/trn_rl_repo"):
    if os.path.isdir(_p) and _p not in sys.path:
        sys.path.insert(0, _p)

import concourse.bass as bass
import concourse.mybir as mybir
import concourse.tile as tile

BF16 = mybir.dt.bfloat16
F32 = mybir.dt.float32
AF = mybir.ActivationFunctionType
ALU = mybir.AluOpType

B, CIN, H, W = 8, 256, 96, 96
EMB1, EMB2 = 256, 128
NCLS, NEG, NLAY = 80, 3, 3
HP, WP = H + 2, W + 2  # zero-padded image
NPIX = H * W
N_CORES = 8

# row blocks: 19 blocks of 5 rows + 1 block of 1 row (5*96=480 <= 512 PSUM bank)
BLOCKS = [(k * 5, 5) for k in range(19)] + [(95, 1)]
# superblocks pair consecutive blocks for the wide elementwise epilogue
SUPER = [(2 * s, 2 * s + 1) for s in range(10)]


# ---------------------------------------------------------------------------
# TileContext drain patch: this walrus build rejects >N sem waits on one
# TPB_CTRL instruction ("Too many sync wait commands"). Absorb the final
# drain's waits into a chain of sync-engine NOPs (one proc each) instead.
# ---------------------------------------------------------------------------
def _patch_tile_drain():
    if getattr(tile.TileContext, "_drain_patched", False):
        return
    from concourse.vector_clock import ScopedClock, VectorClock
    from concourse.tile_sem_assignment import N_PROCS

    # Split instructions carrying >2 embedded sem waits (walrus rejects them):
    # spill the excess onto same-engine NoOps committed immediately before.
    _orig_add = tile.TileContext._add_instruction

    def _add_split(self, inst):
        si = inst.sync_info
        if si is not None and si.on_wait and len(si.on_wait) > 1:
            waits = list(si.on_wait)
            excess, keep = waits[:-1], waits[-1:]
            for i, w in enumerate(excess):
                nop = mybir.InstNoOp(name=f"{inst.name}-wsplit{i}", ins=[], outs=[])
                nop.engine = inst.engine
                nop.sync_info = mybir.SyncInfo(on_wait=[w], on_update=[])
                _orig_add(self, nop)
            inst.sync_info = mybir.SyncInfo(on_wait=keep, on_update=list(si.on_update))
        _orig_add(self, inst)

    tile.TileContext._add_instruction = _add_split

    def _patched(self, tick_clock, wait_clock):
        gc = tick_clock.global_clock
        for p in range(N_PROCS):
            if gc[p] == 0:
                continue
            partial = VectorClock([gc[q] if q == p else 0 for q in range(N_PROCS)])
            nop_inst = self.nc.sync.nop()
            wait_clock.add_sem_waits(nop_inst.ins, ScopedClock({None: partial}))
        self.nc.sync.drain()
        self.nc.all_engine_barrier()
        assert self.sems is not None
        popped = self.nc._tile_sem_poison_stack.pop()
        assert popped is self._sem_poison
        self.nc.clear_and_free_semaphores(list(self.sems.allocated().values()))
        self.nc.all_engine_barrier()

    tile.TileContext._drain_and_barrier = _patched
    tile.TileContext._drain_patched = True


def build_nc() -> bass.Bass:
    _patch_tile_drain()
    nc = bass.Bass()

    # extra activation-bias constants (only 0.0/1.0 are pre-registered)
    for val in (2.0, -4.0, 0.6):
        t = nc.alloc_sbuf_tensor(f"const-f32-{val}", [128, 1], F32)
        nc.gpsimd.memset(t.ap(), val)
        nc.const_aps.aps[(F32, val)] = t.ap()
    nc.all_engine_barrier()

    xp = nc.declare_dram_parameter("xp", [2, 128, HP * WP], BF16, isOutput=False)
    w1p = nc.declare_dram_parameter("w1p", [2, 128, 9 * 2 * 128], BF16, isOutput=False)
    w2p = nc.declare_dram_parameter("w2p", [2, 128, 9 * 128], BF16, isOutput=False)
    ptp = nc.declare_dram_parameter("ptp", [128, 4 * NCLS], BF16, isOutput=False)
    b1p = nc.declare_dram_parameter("b1p", [128, 2], F32, isOutput=False)
    b2p = nc.declare_dram_parameter("b2p", [128, 1], F32, isOutput=False)

    cls_o = nc.declare_dram_parameter("cls_o", [NCLS, NPIX], F32, isOutput=True)
    csn_o = nc.declare_dram_parameter("csn_o", [NCLS, NPIX], F32, isOutput=True)
    dst_o = nc.declare_dram_parameter("dst_o", [NCLS, NPIX], F32, isOutput=True)
    dsn_o = nc.declare_dram_parameter("dsn_o", [NCLS, NEG, NPIX], F32, isOutput=True)
    po_o = nc.declare_dram_parameter("po_o", [NCLS, NPIX], F32, isOutput=True)

    with tile.TileContext(nc) as tc:
        with (
            tc.tile_pool(name="persist", bufs=1) as pp,
            tc.tile_pool(name="work", bufs=1) as wk,
            tc.tile_pool(name="psA", bufs=1, space="PSUM") as psA,
            tc.tile_pool(name="psB", bufs=1, space="PSUM") as psB,
            tc.tile_pool(name="psD", bufs=1, space="PSUM") as psD,
        ):
            # ---- persistent tiles + input DMA ----
            xpad = [pp.tile([128, HP, WP], BF16, name=f"xpad{i}") for i in range(2)]
            hpad = [pp.tile([128, HP, WP], BF16, name=f"hpad{i}") for i in range(2)]
            embn = pp.tile([128, H, W], BF16, name="embn")
            w1 = [pp.tile([128, 9 * 2 * 128], BF16, name=f"w1_{i}") for i in range(2)]
            w2 = [pp.tile([128, 9 * 128], BF16, name=f"w2_{i}") for i in range(2)]
            pt = pp.tile([128, 4 * NCLS], BF16, name="pt")
            b1 = pp.tile([128, 2], F32, name="b1")
            b2 = pp.tile([128, 1], F32, name="b2")
            ones128 = pp.tile([128, 1], BF16, name="ones128")
            ones1 = pp.tile([1, 128], BF16, name="ones1")

            for i in range(2):
                nc.sync.dma_start(out=xpad[i][:], in_=xp[i])
                nc.sync.dma_start(out=w1[i][:], in_=w1p[i])
                nc.sync.dma_start(out=w2[i][:], in_=w2p[i])
                # on DVE: keeps the conv1 bias-add (also DVE) from needing a
                # third cross-engine sem wait (walrus caps embedded waits at 2)
                nc.vector.memset(hpad[i][:], 0.0)
            nc.sync.dma_start(out=pt[:], in_=ptp[:])
            nc.sync.dma_start(out=b1[:], in_=b1p[:])
            nc.sync.dma_start(out=b2[:], in_=b2p[:])
            nc.vector.memset(ones128[:], 1.0)
            nc.vector.memset(ones1[:], 1.0)

            # ---- phase helpers ----
            def conv1_block(y0, R):
                # h[:, y0:y0+R, :] for both co tiles, written into hpad interior
                for cot in range(2):
                    ph = psA.tile([128, 5, 96], F32, tag=f"ph{cot}", name=f"ph{cot}")
                    k = 0
                    for tap in range(9):
                        ty, tx = divmod(tap, 3)
                        for cit in range(2):
                            nc.tensor.matmul(
                                out=ph[:, :R, :],
                                lhsT=w1[cit][:, (tap * 2 + cot) * 128:(tap * 2 + cot + 1) * 128],
                                rhs=xpad[cit][:, y0 + ty:y0 + ty + R, tx:tx + 96],
                                start=(k == 0),
                                stop=(k == 17),
                            )
                            k += 1
                    nc.vector.tensor_scalar_add(
                        out=hpad[cot][:, y0 + 1:y0 + 1 + R, 1:97],
                        in0=ph[:, :R, :],
                        scalar1=b1[:, cot:cot + 1],
                    )

            def conv2_block(y0, R):
                # normalized embedding into embn[:, y0:y0+R, :]
                pe = psB.tile([128, 5, 96], F32, tag="pe", name="pe")
                k = 0
                for tap in range(9):
                    ty, tx = divmod(tap, 3)
                    for cit in range(2):
                        nc.tensor.matmul(
                            out=pe[:, :R, :],
                            lhsT=w2[cit][:, tap * 128:(tap + 1) * 128],
                            rhs=hpad[cit][:, y0 + ty:y0 + ty + R, tx:tx + 96],
                            start=(k == 0),
                            stop=(k == 17),
                        )
                        k += 1
                embb = wk.tile([128, 5, 96], F32, tag="embb", bufs=2, name="embb")
                nc.vector.tensor_scalar_add(
                    out=embb[:, :R, :], in0=pe[:, :R, :], scalar1=b2[:, 0:1]
                )
                sq = wk.tile([128, 5, 96], BF16, tag="sq", bufs=2, name="sq")
                nc.vector.tensor_mul(
                    out=sq[:, :R, :], in0=embb[:, :R, :], in1=embb[:, :R, :]
                )
                pss = psB.tile([1, 5, 96], F32, tag="pss", name="pss")
                nc.tensor.matmul(
                    out=pss[:, :R, :], lhsT=ones128[:, 0:1], rhs=sq[:, :R, :]
                )
                std = wk.tile([1, 5, 96], F32, tag="std", bufs=2, name="std")
                nc.scalar.activation(out=std[:, :R, :], in_=pss[:, :R, :], func=AF.Sqrt)
                rn = wk.tile([1, 5, 96], F32, tag="rn", bufs=2, name="rn")
                nc.vector.reciprocal(out=rn[:, :R, :], in_=std[:, :R, :])
                rnb = wk.tile([1, 5, 96], BF16, tag="rnb", bufs=2, name="rnb")
                nc.vector.tensor_copy(out=rnb[:, :R, :], in_=rn[:, :R, :])
                # broadcast rn across partitions via PE outer product
                prb = psB.tile([128, 5, 96], F32, tag="prb", name="prb")
                nc.tensor.matmul(
                    out=prb[:, :R, :], lhsT=ones1[0:1, :], rhs=rnb[0:1, :R, :]
                )
                nc.vector.tensor_mul(
                    out=embn[:, y0:y0 + R, :], in0=embb[:, :R, :], in1=prb[:, :R, :]
                )

            def dots_and_epilogue(s):
                kblks = [BLOCKS[j] for j in SUPER[s]]
                ys0 = kblks[0][0]
                RS = sum(r for _, r in kblks)
                c0 = ys0 * 96  # column offset in [NCLS, NPIX] outputs

                dist = wk.tile([NCLS, 10, 96], F32, tag="dist", bufs=2, name="dist")
                dn = [
                    wk.tile([NCLS, 10, 96], F32, tag=f"dn{m}", bufs=2, name=f"dn{m}")
                    for m in range(3)
                ]
                po = wk.tile([NCLS, 10, 96], F32, tag="po", bufs=2, name="po")

                for (y0, R) in kblks:
                    r0 = y0 - ys0  # row offset within superblock tiles
                    for j in range(4):
                        pd = psD.tile([NCLS, 5, 96], F32, tag="dot", bufs=3, name="pd")
                        nc.tensor.matmul(
                            out=pd[:, :R, :],
                            lhsT=pt[:, j * NCLS:(j + 1) * NCLS],
                            rhs=embn[:, y0:y0 + R, :],
                        )
                        if j == 0:
                            # distances = sqrt(2 - 2*dot); probs_ori = exp(4*dot-4)
                            nc.scalar.activation(
                                out=dist[:, r0:r0 + R, :], in_=pd[:, :R, :],
                                func=AF.Sqrt, scale=-2.0, bias=2.0,
                            )
                            nc.scalar.activation(
                                out=po[:, r0:r0 + R, :], in_=pd[:, :R, :],
                                func=AF.Exp, scale=4.0, bias=-4.0,
                            )
                        else:
                            nc.scalar.activation(
                                out=dn[j - 1][:, r0:r0 + R, :], in_=pd[:, :R, :],
                                func=AF.Sqrt, scale=-2.0, bias=2.0,
                            )

                sl = (slice(None), slice(0, RS), slice(None))
                # DMA the raw distance outputs
                nc.sync.dma_start(out=dst_o[:, c0:c0 + RS * 96], in_=dist[sl])
                for m in range(3):
                    nc.sync.dma_start(
                        out=dsn_o[:, m, c0:c0 + RS * 96], in_=dn[m][sl]
                    )
                nc.sync.dma_start(out=po_o[:, c0:c0 + RS * 96], in_=po[sl])

                # min over m of distances_neg
                mn = wk.tile([NCLS, 10, 96], F32, tag="mn", bufs=2, name="mn")
                nc.vector.tensor_tensor(out=mn[sl], in0=dn[0][sl], in1=dn[1][sl], op=ALU.min)
                nc.vector.tensor_tensor(out=mn[sl], in0=mn[sl], in1=dn[2][sl], op=ALU.min)
                # cls_score_neg = exp(-2*mn^2)
                sqn = wk.tile([NCLS, 10, 96], F32, tag="sqn", bufs=2, name="sqn")
                nc.scalar.activation(out=sqn[sl], in_=mn[sl], func=AF.Square)
                nc.scalar.activation(out=sqn[sl], in_=sqn[sl], func=AF.Exp, scale=-2.0)
                nc.sync.dma_start(out=csn_o[:, c0:c0 + RS * 96], in_=sqn[sl])
                # u = relu(0.6 - 0.3*mn) = 0.3*relu(2 - mn)
                nc.scalar.activation(out=mn[sl], in_=mn[sl], func=AF.Relu, scale=-0.3, bias=0.6)
                # shifted = dist + u (in place over dist); s2 = shifted^2
                nc.vector.tensor_add(out=dist[sl], in0=dist[sl], in1=mn[sl])
                nc.scalar.activation(out=dist[sl], in_=dist[sl], func=AF.Square)
                # p = exp(-2*s2)
                p = wk.tile([NCLS, 10, 96], F32, tag="p", bufs=2, name="p")
                nc.scalar.activation(out=p[sl], in_=dist[sl], func=AF.Exp, scale=-2.0)
                # q = 1 - p  (before p gets clipped in place)
                q = wk.tile([NCLS, 10, 96], F32, tag="q", bufs=2, name="q")
                nc.vector.tensor_scalar(
                    out=q[sl], in0=p[sl], scalar1=-1.0, scalar2=1.0,
                    op0=ALU.mult, op1=ALU.add,
                )
                nc.vector.tensor_scalar_max(out=p[sl], in0=p[sl], scalar1=1e-5)
                nc.scalar.activation(out=p[sl], in_=p[sl], func=AF.Ln)
                nc.vector.tensor_scalar_max(out=q[sl], in0=q[sl], scalar1=1e-5)
                nc.scalar.activation(out=q[sl], in_=q[sl], func=AF.Ln)
                nc.vector.tensor_sub(out=q[sl], in0=p[sl], in1=q[sl])
                nc.sync.dma_start(out=cls_o[:, c0:c0 + RS * 96], in_=q[sl])

            # ---- interleaved schedule: conv1 runs 2 blocks ahead of conv2 ----
            conv1_block(*BLOCKS[0])
            conv1_block(*BLOCKS[1])
            for s in range(len(SUPER)):
                for j in (2 * s + 2, 2 * s + 3):
                    if j < len(BLOCKS):
                        conv1_block(*BLOCKS[j])
                conv2_block(*BLOCKS[2 * s])
                conv2_block(*BLOCKS[2 * s + 1])
                dots_and_epilogue(s)

    # this walrus build rejects instructions with >1 embedded sem wait;
    # fail fast here instead of minutes later inside neuronx-cc
    for name, inst in nc.inst_map.items():
        si = inst.sync_info
        nw = len(si.on_wait) if si and si.on_wait else 0
        assert nw <= 1, f"{name} ({type(inst).__name__}) has {nw} sem waits"
    return nc


def _l2norm_np(v, axis):
    n = np.maximum(np.linalg.norm(v, axis=axis, keepdims=True), 1e-12)
    return v / n


def _prep_host(x, conv1_w, conv1_b, bn_gamma, bn_beta, bn_mean, bn_var,
               conv2_w, conv2_b, rep_w, rep_b, neg_w, neg_b):
    """Host-side packing: BN fold, weight layout, prototype MLPs, input pad."""
    f32 = np.float32
    x = np.asarray(x, f32)
    conv1_w = np.asarray(conv1_w, f32)
    conv1_b = np.asarray(conv1_b, f32)
    scale = np.asarray(bn_gamma, f32) / np.sqrt(np.asarray(bn_var, f32) + 1e-5)
    w1f = conv1_w * scale[:, None, None, None]
    b1f = (conv1_b - np.asarray(bn_mean, f32)) * scale + np.asarray(bn_beta, f32)

    # conv1 weights -> [cit, ci, (ky kx cot co)]
    w1r = w1f.reshape(2, 128, 2, 128, 3, 3)  # [cot, co, cit, ci, ky, kx]
    w1p = np.ascontiguousarray(
        w1r.transpose(2, 3, 4, 5, 0, 1).reshape(2, 128, 9 * 2 * 128)
    ).astype(ml_dtypes.bfloat16)
    # conv2 weights -> [cit, ci, (ky kx co)]
    w2r = np.asarray(conv2_w, f32).reshape(128, 2, 128, 3, 3)  # [co, cit, ci, ky, kx]
    w2p = np.ascontiguousarray(
        w2r.transpose(1, 2, 3, 4, 0).reshape(2, 128, 9 * 128)
    ).astype(ml_dtypes.bfloat16)

    b1p = np.ascontiguousarray(b1f.reshape(2, 128).T)  # [co_within, cot]
    b2p = np.asarray(conv2_b, f32).reshape(128, 1).copy()

    # prototypes (tiny) on host, fp32
    reps = (np.asarray(rep_w, f32)[:, 0] + np.asarray(rep_b, f32)).reshape(NCLS, EMB2)
    r = _l2norm_np(reps, 1)  # [C, E]
    nw = np.asarray(neg_w, f32)
    nb = np.asarray(neg_b, f32)
    rn = np.empty((NEG, NCLS, EMB2), f32)
    for m in range(NEG):
        h = r
        for i in range(NLAY):
            h = h @ nw[m, i].T + nb[m, i]
            if i < NLAY - 1:
                h = np.maximum(h, 0.0)
        rn[m] = h
    rn = _l2norm_np(np.transpose(rn, (1, 0, 2)), 2)  # [C, NEG, E]

    ptp = np.zeros((128, 4 * NCLS), f32)
    ptp[:, :NCLS] = r.T
    for m in range(NEG):
        ptp[:, NCLS + m * NCLS:NCLS + (m + 1) * NCLS] = rn[:, m, :].T
    ptp = ptp.astype(ml_dtypes.bfloat16)

    # per-image padded bf16 input [2, 128, HP*WP]
    xpads = []
    for b in range(B):
        xi = np.zeros((2, 128, HP, WP), f32)
        xi[:, :, 1:97, 1:97] = x[b].reshape(2, 128, H, W)
        xpads.append(xi.reshape(2, 128, HP * WP).astype(ml_dtypes.bfloat16))

    shared = {"w1p": w1p, "w2p": w2p, "ptp": ptp, "b1p": b1p, "b2p": b2p}
    return xpads, shared


_NC_CACHE = {}


def kernel(**inputs) -> tuple:
    from concourse.bass_utils import run_bass_kernel_spmd

    xpads, shared = _prep_host(**inputs)
    if "nc" not in _NC_CACHE:
        _NC_CACHE["nc"] = build_nc()
    nc = _NC_CACHE["nc"]

    in_maps = [dict(shared, xp=xpads[b]) for b in range(B)]
    res = run_bass_kernel_spmd(nc, in_maps, list(range(N_CORES)))

    f32 = np.float32
    cls = np.stack([np.asarray(res.results[b]["cls_o"], f32).reshape(NCLS, H, W) for b in range(B)])
    csn = np.stack([np.asarray(res.results[b]["csn_o"], f32).reshape(NCLS, H, W) for b in range(B)])
    dst = np.stack([np.asarray(res.results[b]["dst_o"], f32).reshape(NCLS, 1, H, W) for b in range(B)])
    dsn = np.stack([np.asarray(res.results[b]["dsn_o"], f32).reshape(NCLS, NEG, H, W) for b in range(B)])
    po = np.stack([np.asarray(res.results[b]["po_o"], f32).reshape(NCLS, H, W) for b in range(B)])
    return (cls, csn, dst, dsn, po)


# revision 13
# speedup vs baseline: 1.1100x; 1.1100x over previous
"""Trainium2 Bass kernel for nn_FCOSDMLNegHead2 (vq_codebook).

Sharding: data-parallel over batch B=8 across 8 NeuronCores (1 image/core).
Per core: conv1 3x3 (256->256, BN folded on host) + conv2 3x3 (256->128) as
9-tap matmuls in bf16, per-pixel L2 norm, dot with 320 host-precomputed
prototypes, elementwise epilogue producing the 5 outputs.

ACT-table discipline: Sqrt lives in a different ACT table set than
{Exp, Ln, Relu, Square}; the epilogue is ordered so each superblock does
all its Sqrt work first, then all Exp/Ln/Relu/Square work, keeping table
reloads to ~2 per superblock.
"""

import os
import sys

import numpy as np
import ml_dtypes

for _p in ("/opt/trn_rl_repo", "/root/.axon_site/_ro/trn_rl_repo"):
    if os.path.isdir(_p) and _p not in sys.path:
        sys.path.insert(0, _p)

import concourse.bass as bass
import concourse.mybir as mybir
import concourse.tile as tile

BF16 = mybir.dt.bfloat16
F32 = mybir.dt.float32
AF = mybir.ActivationFunctionType
ALU = mybir.AluOpType

B, CIN, H, W = 8, 256, 96, 96
EMB1, EMB2 = 256, 128
NCLS, NEG, NLAY = 80, 3, 3
HP, WP = H + 2, W + 2  # zero-padded image
NPIX = H * W
N_CORES = 8

# conv row blocks: 19 blocks of 5 rows + 1 of 1 row (5*96=480 <= 512 PSUM bank)
BLOCKS = [(k * 5, 5) for k in range(19)] + [(95, 1)]
# dot/epilogue pixel chunks: 18 chunks of 512 pixels, paired into 9 superblocks
NCHUNK, CW = 18, 512
NSUP, SW = 9, 1024


# ---------------------------------------------------------------------------
# Tile patches for this walrus build:
#  * instructions may carry at most ONE embedded sem wait -> spill extras
#    onto same-engine NoOps committed just before the instruction
#  * the kernel-tail Drain may carry none -> absorb into sync-engine NOPs
# ---------------------------------------------------------------------------
def _patch_tile_drain():
    if getattr(tile.TileContext, "_drain_patched", False):
        return
    from concourse.vector_clock import ScopedClock, VectorClock
    from concourse.tile_sem_assignment import N_PROCS

    _orig_add = tile.TileContext._add_instruction

    def _add_split(self, inst):
        si = inst.sync_info
        if si is not None and si.on_wait and len(si.on_wait) > 1:
            waits = list(si.on_wait)
            excess, keep = waits[:-1], waits[-1:]
            for i, w in enumerate(excess):
                nop = mybir.InstNoOp(name=f"{inst.name}-wsplit{i}", ins=[], outs=[])
                nop.engine = inst.engine
                nop.sync_info = mybir.SyncInfo(on_wait=[w], on_update=[])
                _orig_add(self, nop)
            inst.sync_info = mybir.SyncInfo(on_wait=keep, on_update=list(si.on_update))
        _orig_add(self, inst)

    tile.TileContext._add_instruction = _add_split

    def _patched(self, tick_clock, wait_clock):
        gc = tick_clock.global_clock
        for p in range(N_PROCS):
            if gc[p] == 0:
                continue
            partial = VectorClock([gc[q] if q == p else 0 for q in range(N_PROCS)])
            nop_inst = self.nc.sync.nop()
            wait_clock.add_sem_waits(nop_inst.ins, ScopedClock({None: partial}))
        self.nc.sync.drain()
        self.nc.all_engine_barrier()
        assert self.sems is not None
        popped = self.nc._tile_sem_poison_stack.pop()
        assert popped is self._sem_poison
        self.nc.clear_and_free_semaphores(list(self.sems.allocated().values()))
        self.nc.all_engine_barrier()

    tile.TileContext._drain_and_barrier = _patched
    tile.TileContext._drain_patched = True


def build_nc() -> bass.Bass:
    _patch_tile_drain()
    nc = bass.Bass()

    # extra activation-bias constants (only 0.0/1.0 are pre-registered)
    for val in (2.0, -4.0, 0.6):
        t = nc.alloc_sbuf_tensor(f"const-f32-{val}", [128, 1], F32)
        nc.gpsimd.memset(t.ap(), val)
        nc.const_aps.aps[(F32, val)] = t.ap()
    nc.all_engine_barrier()

    xp = nc.declare_dram_parameter("xp", [2, 128, HP * WP], BF16, isOutput=False)
    w1p = nc.declare_dram_parameter("w1p", [2, 128, 9 * 2 * 128], BF16, isOutput=False)
    w2p = nc.declare_dram_parameter("w2p", [2, 128, 9 * 128], BF16, isOutput=False)
    ptp = nc.declare_dram_parameter("ptp", [128, 4 * NCLS], BF16, isOutput=False)
    b1p = nc.declare_dram_parameter("b1p", [128, 2], F32, isOutput=False)
    b2p = nc.declare_dram_parameter("b2p", [128, 1], F32, isOutput=False)

    cls_o = nc.declare_dram_parameter("cls_o", [NCLS, NPIX], F32, isOutput=True)
    csn_o = nc.declare_dram_parameter("csn_o", [NCLS, NPIX], F32, isOutput=True)
    dst_o = nc.declare_dram_parameter("dst_o", [NCLS, NPIX], F32, isOutput=True)
    dsn_o = nc.declare_dram_parameter("dsn_o", [NCLS, NEG, NPIX], F32, isOutput=True)
    po_o = nc.declare_dram_parameter("po_o", [NCLS, NPIX], F32, isOutput=True)

    with tile.TileContext(nc) as tc:
        with (
            tc.tile_pool(name="persist", bufs=1) as pp,
            tc.tile_pool(name="work", bufs=1) as wk,
            tc.tile_pool(name="psA", bufs=1, space="PSUM") as psA,
            tc.tile_pool(name="psB", bufs=1, space="PSUM") as psB,
            tc.tile_pool(name="psD", bufs=1, space="PSUM") as psD,
        ):
            # ---- persistent tiles + input DMA (weights first, x row-chunked
            # so conv1 can start before the whole image lands) ----
            xpad = [pp.tile([128, HP, WP], BF16, name=f"xpad{i}") for i in range(2)]
            hpad = [pp.tile([128, HP, WP], BF16, name=f"hpad{i}") for i in range(2)]
            embn = pp.tile([128, NPIX], BF16, name="embn")
            w1 = [pp.tile([128, 9 * 2 * 128], BF16, name=f"w1_{i}") for i in range(2)]
            w2 = [pp.tile([128, 9 * 128], BF16, name=f"w2_{i}") for i in range(2)]
            pt = pp.tile([128, 4 * NCLS], BF16, name="pt")
            b1 = pp.tile([128, 2], F32, name="b1")
            b2 = pp.tile([128, 1], F32, name="b2")
            ones128 = pp.tile([128, 1], BF16, name="ones128")
            ones1 = pp.tile([1, 128], BF16, name="ones1")

            for i in range(2):
                nc.sync.dma_start(out=w1[i][:], in_=w1p[i])
            XCH = [(0, 25), (25, 50), (50, 75), (75, 98)]
            for r0, r1 in XCH:
                for i in range(2):
                    nc.sync.dma_start(
                        out=xpad[i][:, r0:r1, :], in_=xp[i, :, r0 * WP:r1 * WP]
                    )
            for i in range(2):
                nc.sync.dma_start(out=w2[i][:], in_=w2p[i])
            nc.sync.dma_start(out=pt[:], in_=ptp[:])
            nc.sync.dma_start(out=b1[:], in_=b1p[:])
            nc.sync.dma_start(out=b2[:], in_=b2p[:])
            nc.vector.memset(ones128[:], 1.0)
            nc.vector.memset(ones1[:], 1.0)
            # hpad border zeros only (interior is fully overwritten by conv1)
            for i in range(2):
                nc.gpsimd.memset(hpad[i][:, 0:1, :], 0.0)
                nc.gpsimd.memset(hpad[i][:, HP - 1:HP, :], 0.0)
                nc.gpsimd.memset(hpad[i][:, 1:HP - 1, 0:1], 0.0)
                nc.gpsimd.memset(hpad[i][:, 1:HP - 1, WP - 1:WP], 0.0)

            def conv1_block(y0, R):
                for cot in range(2):
                    ph = psA.tile([128, 5, 96], F32, tag=f"ph{cot}", name=f"ph{cot}")
                    k = 0
                    for tap in range(9):
                        ty, tx = divmod(tap, 3)
                        for cit in range(2):
                            nc.tensor.matmul(
                                out=ph[:, :R, :],
                                lhsT=w1[cit][:, (tap * 2 + cot) * 128:(tap * 2 + cot + 1) * 128],
                                rhs=xpad[cit][:, y0 + ty:y0 + ty + R, tx:tx + 96],
                                start=(k == 0),
                                stop=(k == 17),
                            )
                            k += 1
                    nc.vector.tensor_scalar_add(
                        out=hpad[cot][:, y0 + 1:y0 + 1 + R, 1:97],
                        in0=ph[:, :R, :],
                        scalar1=b1[:, cot:cot + 1],
                    )

            def conv2_block(y0, R):
                # normalized embedding into embn[:, y0*96 : (y0+R)*96]
                pe = psB.tile([128, 5, 96], F32, tag="pe", name="pe")
                k = 0
                for tap in range(9):
                    ty, tx = divmod(tap, 3)
                    for cit in range(2):
                        nc.tensor.matmul(
                            out=pe[:, :R, :],
                            lhsT=w2[cit][:, tap * 128:(tap + 1) * 128],
                            rhs=hpad[cit][:, y0 + ty:y0 + ty + R, tx:tx + 96],
                            start=(k == 0),
                            stop=(k == 17),
                        )
                        k += 1
                embb = wk.tile([128, 5, 96], F32, tag="embb", bufs=2, name="embb")
                nc.vector.tensor_scalar_add(
                    out=embb[:, :R, :], in0=pe[:, :R, :], scalar1=b2[:, 0:1]
                )
                sq = wk.tile([128, 5, 96], BF16, tag="sq", bufs=2, name="sq")
                nc.vector.tensor_mul(
                    out=sq[:, :R, :], in0=embb[:, :R, :], in1=embb[:, :R, :]
                )
                pss = psB.tile([1, 5, 96], F32, tag="pss", name="pss")
                nc.tensor.matmul(
                    out=pss[:, :R, :], lhsT=ones128[:, 0:1], rhs=sq[:, :R, :]
                )
                std = wk.tile([1, 5, 96], F32, tag="std", bufs=2, name="std")
                nc.scalar.activation(out=std[:, :R, :], in_=pss[:, :R, :], func=AF.Sqrt)
                nc.vector.tensor_scalar_max(
                    out=std[:, :R, :], in0=std[:, :R, :], scalar1=1e-12
                )
                rn = wk.tile([1, 5, 96], F32, tag="rn", bufs=2, name="rn")
                nc.vector.reciprocal(out=rn[:, :R, :], in_=std[:, :R, :])
                rnb = wk.tile([1, 5, 96], BF16, tag="rnb", bufs=2, name="rnb")
                nc.vector.tensor_copy(out=rnb[:, :R, :], in_=rn[:, :R, :])
                # broadcast rn across partitions via PE outer product
                prb = psB.tile([128, 5, 96], F32, tag="prb", name="prb")
                nc.tensor.matmul(
                    out=prb[:, :R, :], lhsT=ones1[0:1, :], rhs=rnb[0:1, :R, :]
                )
                nc.vector.tensor_mul(
                    out=embn[:, y0 * 96:(y0 + R) * 96],
                    in0=embb[:, :R, :],
                    in1=prb[:, :R, :],
                )

            def dots_super(s):
                c0 = s * SW  # pixel offset
                dist = wk.tile([NCLS, SW], F32, tag="dist", bufs=2, name="dist")
                dn = [
                    wk.tile([NCLS, SW], F32, tag=f"dn{m}", bufs=2, name=f"dn{m}")
                    for m in range(3)
                ]
                # ---- sqrt-table phase: dot matmuls + distance sqrt ----
                for half in range(2):
                    p0 = c0 + half * CW
                    for j in range(4):
                        pd = psD.tile([NCLS, CW], F32, tag="dot", bufs=3, name="pd")
                        nc.tensor.matmul(
                            out=pd[:, :],
                            lhsT=pt[:, j * NCLS:(j + 1) * NCLS],
                            rhs=embn[:, p0:p0 + CW],
                        )
                        dstt = dist if j == 0 else dn[j - 1]
                        nc.scalar.activation(
                            out=dstt[:, half * CW:(half + 1) * CW], in_=pd[:, :],
                            func=AF.Sqrt, scale=-2.0, bias=2.0,
                        )
                nc.sync.dma_start(out=dst_o[:, c0:c0 + SW], in_=dist[:, :])
                for m in range(3):
                    nc.sync.dma_start(out=dsn_o[:, m, c0:c0 + SW], in_=dn[m][:, :])

                # ---- ln/exp/relu/square-table phase ----
                tmp = wk.tile([NCLS, SW], F32, tag="tmp", bufs=2, name="tmp")
                po = wk.tile([NCLS, SW], F32, tag="po", bufs=2, name="po")
                # probs_ori = exp(-2*dist^2)
                nc.scalar.activation(out=tmp[:, :], in_=dist[:, :], func=AF.Square)
                nc.scalar.activation(out=po[:, :], in_=tmp[:, :], func=AF.Exp, scale=-2.0)
                nc.sync.dma_start(out=po_o[:, c0:c0 + SW], in_=po[:, :])
                # min over m of distances_neg  (gpsimd: SBUF-only elementwise)
                mn = wk.tile([NCLS, SW], F32, tag="mn", bufs=2, name="mn")
                nc.vector.tensor_tensor(out=mn[:, :], in0=dn[0][:, :], in1=dn[1][:, :], op=ALU.min)
                nc.vector.tensor_tensor(out=mn[:, :], in0=mn[:, :], in1=dn[2][:, :], op=ALU.min)
                # cls_score_neg = exp(-2*mn^2)
                nc.scalar.activation(out=tmp[:, :], in_=mn[:, :], func=AF.Square)
                nc.scalar.activation(out=tmp[:, :], in_=tmp[:, :], func=AF.Exp, scale=-2.0)
                nc.sync.dma_start(out=csn_o[:, c0:c0 + SW], in_=tmp[:, :])
                # u = relu(0.6 - 0.3*mn); shifted = dist + u; s2 = shifted^2
                nc.scalar.activation(out=mn[:, :], in_=mn[:, :], func=AF.Relu, scale=-0.3, bias=0.6)
                nc.vector.tensor_add(out=dist[:, :], in0=dist[:, :], in1=mn[:, :])
                nc.scalar.activation(out=dist[:, :], in_=dist[:, :], func=AF.Square)
                p = wk.tile([NCLS, SW], F32, tag="p", bufs=2, name="p")
                nc.scalar.activation(out=p[:, :], in_=dist[:, :], func=AF.Exp, scale=-2.0)
                q = wk.tile([NCLS, SW], F32, tag="q", bufs=2, name="q")
                nc.vector.tensor_scalar(
                    out=q[:, :], in0=p[:, :], scalar1=-1.0, scalar2=1.0,
                    op0=ALU.mult, op1=ALU.add,
                )
                nc.vector.tensor_scalar_max(out=p[:, :], in0=p[:, :], scalar1=1e-5)
                nc.scalar.activation(out=p[:, :], in_=p[:, :], func=AF.Ln)
                nc.vector.tensor_scalar_max(out=q[:, :], in0=q[:, :], scalar1=1e-5)
                nc.scalar.activation(out=q[:, :], in_=q[:, :], func=AF.Ln)
                nc.vector.tensor_sub(out=q[:, :], in0=p[:, :], in1=q[:, :])
                nc.sync.dma_start(out=cls_o[:, c0:c0 + SW], in_=q[:, :])

            # ---- interleaved schedule ----
            n_c1 = 0
            n_b1 = 0

            def emit_b1_upto(target):
                nonlocal n_c1, n_b1
                while n_b1 < target:
                    while n_c1 < min(len(BLOCKS), n_b1 + 2):
                        conv1_block(*BLOCKS[n_c1])
                        n_c1 += 1
                    conv2_block(*BLOCKS[n_b1])
                    n_b1 += 1

            for s in range(NSUP):
                need_rows = min(H, (s + 1) * SW // W + 1)
                emit_b1_upto(min(len(BLOCKS), (need_rows + 4) // 5))
                dots_super(s)

    # this walrus build rejects instructions with >1 embedded sem wait;
    # fail fast here instead of minutes later inside neuronx-cc
    for name, inst in nc.inst_map.items():
        si = inst.sync_info
        nw = len(si.on_wait) if si and si.on_wait else 0
        assert nw <= 1, f"{name} ({type(inst).__name__}) has {nw} sem waits"
    return nc


def _l2norm_np(v, axis):
    n = np.maximum(np.linalg.norm(v, axis=axis, keepdims=True), 1e-12)
    return v / n


def _prep_host(x, conv1_w, conv1_b, bn_gamma, bn_beta, bn_mean, bn_var,
               conv2_w, conv2_b, rep_w, rep_b, neg_w, neg_b):
    """Host-side packing: BN fold, weight layout, prototype MLPs, input pad."""
    f32 = np.float32
    x = np.asarray(x, f32)
    conv1_w = np.asarray(conv1_w, f32)
    conv1_b = np.asarray(conv1_b, f32)
    scale = np.asarray(bn_gamma, f32) / np.sqrt(np.asarray(bn_var, f32) + 1e-5)
    w1f = conv1_w * scale[:, None, None, None]
    b1f = (conv1_b - np.asarray(bn_mean, f32)) * scale + np.asarray(bn_beta, f32)

    # conv1 weights -> [cit, ci, (ky kx cot co)]
    w1r = w1f.reshape(2, 128, 2, 128, 3, 3)  # [cot, co, cit, ci, ky, kx]
    w1p = np.ascontiguousarray(
        w1r.transpose(2, 3, 4, 5, 0, 1).reshape(2, 128, 9 * 2 * 128)
    ).astype(ml_dtypes.bfloat16)
    # conv2 weights -> [cit, ci, (ky kx co)]
    w2r = np.asarray(conv2_w, f32).reshape(128, 2, 128, 3, 3)  # [co, cit, ci, ky, kx]
    w2p = np.ascontiguousarray(
        w2r.transpose(1, 2, 3, 4, 0).reshape(2, 128, 9 * 128)
    ).astype(ml_dtypes.bfloat16)

    b1p = np.ascontiguousarray(b1f.reshape(2, 128).T)  # [co_within, cot]
    b2p = np.asarray(conv2_b, f32).reshape(128, 1).copy()

    # prototypes (tiny) on host, fp32
    reps = (np.asarray(rep_w, f32)[:, 0] + np.asarray(rep_b, f32)).reshape(NCLS, EMB2)
    r = _l2norm_np(reps, 1)  # [C, E]
    nw = np.asarray(neg_w, f32)
    nb = np.asarray(neg_b, f32)
    rn = np.empty((NEG, NCLS, EMB2), f32)
    for m in range(NEG):
        h = r
        for i in range(NLAY):
            h = h @ nw[m, i].T + nb[m, i]
            if i < NLAY - 1:
                h = np.maximum(h, 0.0)
        rn[m] = h
    rn = _l2norm_np(np.transpose(rn, (1, 0, 2)), 2)  # [C, NEG, E]

    ptp = np.zeros((128, 4 * NCLS), f32)
    ptp[:, :NCLS] = r.T
    for m in range(NEG):
        ptp[:, NCLS + m * NCLS:NCLS + (m + 1) * NCLS] = rn[:, m, :].T
    ptp = ptp.astype(ml_dtypes.bfloat16)

    # per-image padded bf16 input [2, 128, HP*WP]
    xpads = []
    for b in range(B):
        xi = np.zeros((2, 128, HP, WP), f32)
        xi[:, :, 1:97, 1:97] = x[b].reshape(2, 128, H, W)
        xpads.append(xi.reshape(2, 128, HP * WP).astype(ml_dtypes.bfloat16))

    shared = {"w1p": w1p, "w2p": w2p, "ptp": ptp, "b1p": b1p, "b2p": b2p}
    return xpads, shared


_NC_CACHE = {}


def kernel(**inputs) -> tuple:
    from concourse.bass_utils import run_bass_kernel_spmd

    xpads, shared = _prep_host(**inputs)
    if "nc" not in _NC_CACHE:
        _NC_CACHE["nc"] = build_nc()
    nc = _NC_CACHE["nc"]

    in_maps = [dict(shared, xp=xpads[b]) for b in range(B)]
    res = run_bass_kernel_spmd(nc, in_maps, list(range(N_CORES)))

    f32 = np.float32
    cls = np.stack([np.asarray(res.results[b]["cls_o"], f32).reshape(NCLS, H, W) for b in range(B)])
    csn = np.stack([np.asarray(res.results[b]["csn_o"], f32).reshape(NCLS, H, W) for b in range(B)])
    dst = np.stack([np.asarray(res.results[b]["dst_o"], f32).reshape(NCLS, 1, H, W) for b in range(B)])
    dsn = np.stack([np.asarray(res.results[b]["dsn_o"], f32).reshape(NCLS, NEG, H, W) for b in range(B)])
    po = np.stack([np.asarray(res.results[b]["po_o"], f32).reshape(NCLS, H, W) for b in range(B)])
    return (cls, csn, dst, dsn, po)


# revision 18
# speedup vs baseline: 1.2727x; 1.1466x over previous
"""Trainium2 Bass kernel for nn_FCOSDMLNegHead2 (vq_codebook).

Sharding: data-parallel over batch B=8 across 8 NeuronCores (1 image/core).
Per core: conv1 3x3 (256->256, BN folded on host) + conv2 3x3 (256->128) as
9-tap matmuls in bf16, per-pixel L2 norm, dot with 320 host-precomputed
prototypes, elementwise epilogue producing the 5 outputs.

ACT-table discipline: Sqrt lives in a different ACT table set than
{Exp, Ln, Relu, Square}; the epilogue is ordered so each superblock does
all its Sqrt work first, then all Exp/Ln/Relu/Square work, keeping table
reloads to ~2 per superblock.
"""

import os
import sys

import numpy as np
import ml_dtypes

for _p in ("/opt/trn_rl_repo", "/root/.axon_site/_ro/trn_rl_repo"):
    if os.path.isdir(_p) and _p not in sys.path:
        sys.path.insert(0, _p)

import concourse.bass as bass
import concourse.mybir as mybir
import concourse.tile as tile
from concourse.tile import add_dep_helper

BF16 = mybir.dt.bfloat16
F32 = mybir.dt.float32
AF = mybir.ActivationFunctionType
ALU = mybir.AluOpType

B, CIN, H, W = 8, 256, 96, 96
EMB1, EMB2 = 256, 128
NCLS, NEG, NLAY = 80, 3, 3
HP, WP = H + 2, W + 2  # zero-padded image
NPIX = H * W
N_CORES = 8

# conv row blocks: 19 blocks of 5 rows + 1 of 1 row (5*96=480 <= 512 PSUM bank)
BLOCKS = [(k * 5, 5) for k in range(19)] + [(95, 1)]
# dot/epilogue pixel chunks: 18 chunks of 512 pixels, paired into 9 superblocks
NCHUNK, CW = 18, 512
NSUP, SW = 9, 1024


# ---------------------------------------------------------------------------
# Tile patches for this walrus build:
#  * instructions may carry at most ONE embedded sem wait -> spill extras
#    onto same-engine NoOps committed just before the instruction
#  * the kernel-tail Drain may carry none -> absorb into sync-engine NOPs
# ---------------------------------------------------------------------------
def _patch_tile_drain():
    if getattr(tile.TileContext, "_drain_patched", False):
        return
    from concourse.vector_clock import ScopedClock, VectorClock
    from concourse.tile_sem_assignment import N_PROCS

    _orig_add = tile.TileContext._add_instruction

    def _add_split(self, inst):
        si = inst.sync_info
        if si is not None and si.on_wait and len(si.on_wait) > 1:
            waits = list(si.on_wait)
            excess, keep = waits[:-1], waits[-1:]
            for i, w in enumerate(excess):
                nop = mybir.InstNoOp(name=f"{inst.name}-wsplit{i}", ins=[], outs=[])
                nop.engine = inst.engine
                nop.sync_info = mybir.SyncInfo(on_wait=[w], on_update=[])
                _orig_add(self, nop)
            inst.sync_info = mybir.SyncInfo(on_wait=keep, on_update=list(si.on_update))
        _orig_add(self, inst)

    tile.TileContext._add_instruction = _add_split

    def _patched(self, tick_clock, wait_clock):
        gc = tick_clock.global_clock
        for p in range(N_PROCS):
            if gc[p] == 0:
                continue
            partial = VectorClock([gc[q] if q == p else 0 for q in range(N_PROCS)])
            nop_inst = self.nc.sync.nop()
            wait_clock.add_sem_waits(nop_inst.ins, ScopedClock({None: partial}))
        self.nc.sync.drain()
        self.nc.all_engine_barrier()
        assert self.sems is not None
        popped = self.nc._tile_sem_poison_stack.pop()
        assert popped is self._sem_poison
        self.nc.clear_and_free_semaphores(list(self.sems.allocated().values()))
        self.nc.all_engine_barrier()

    tile.TileContext._drain_and_barrier = _patched
    tile.TileContext._drain_patched = True


def build_nc() -> bass.Bass:
    _patch_tile_drain()
    nc = bass.Bass()

    # extra activation-bias constants (only 0.0/1.0 are pre-registered)
    for val in (2.0, -4.0, 0.6):
        t = nc.alloc_sbuf_tensor(f"const-f32-{val}", [128, 1], F32)
        nc.gpsimd.memset(t.ap(), val)
        nc.const_aps.aps[(F32, val)] = t.ap()
    nc.all_engine_barrier()

    xp = nc.declare_dram_parameter("xp", [2, 128, HP * WP], BF16, isOutput=False)
    w1p = nc.declare_dram_parameter("w1p", [2, 128, 9 * 2 * 128], BF16, isOutput=False)
    w2p = nc.declare_dram_parameter("w2p", [2, 128, 9 * 128], BF16, isOutput=False)
    ptp = nc.declare_dram_parameter("ptp", [128, 4 * NCLS], BF16, isOutput=False)
    b1p = nc.declare_dram_parameter("b1p", [128, 2], F32, isOutput=False)
    b2p = nc.declare_dram_parameter("b2p", [128, 1], F32, isOutput=False)

    cls_o = nc.declare_dram_parameter("cls_o", [NCLS, NPIX], F32, isOutput=True)
    csn_o = nc.declare_dram_parameter("csn_o", [NCLS, NPIX], F32, isOutput=True)
    dst_o = nc.declare_dram_parameter("dst_o", [NCLS, NPIX], F32, isOutput=True)
    dsn_o = nc.declare_dram_parameter("dsn_o", [NCLS, NEG, NPIX], F32, isOutput=True)
    po_o = nc.declare_dram_parameter("po_o", [NCLS, NPIX], F32, isOutput=True)

    with tile.TileContext(nc) as tc:
        with (
            tc.tile_pool(name="persist", bufs=1) as pp,
            tc.tile_pool(name="work", bufs=1) as wk,
            tc.tile_pool(name="psA", bufs=1, space="PSUM") as psA,
            tc.tile_pool(name="psB", bufs=1, space="PSUM") as psB,
            tc.tile_pool(name="psD", bufs=1, space="PSUM") as psD,
        ):
            # ---- persistent tiles + input DMA (weights first, x row-chunked
            # so conv1 can start before the whole image lands) ----
            xpad = [pp.tile([128, HP, WP], BF16, name=f"xpad{i}") for i in range(2)]
            hpad = [pp.tile([128, HP, WP], BF16, name=f"hpad{i}") for i in range(2)]
            embn = pp.tile([128, NPIX], BF16, name="embn")
            w1 = [pp.tile([128, 9 * 2 * 128], BF16, name=f"w1_{i}") for i in range(2)]
            w2 = [pp.tile([128, 9 * 128], BF16, name=f"w2_{i}") for i in range(2)]
            pt = pp.tile([128, 4 * NCLS], BF16, name="pt")
            b1 = pp.tile([128, 2], F32, name="b1")
            b2 = pp.tile([128, 1], F32, name="b2")
            ones128 = pp.tile([128, 1], BF16, name="ones128")
            ones1 = pp.tile([1, 128], BF16, name="ones1")

            for i in range(2):
                nc.sync.dma_start(out=w1[i][:], in_=w1p[i])
            XCH = [(0, 25), (25, 50), (50, 75), (75, 98)]
            for r0, r1 in XCH:
                for i in range(2):
                    nc.sync.dma_start(
                        out=xpad[i][:, r0:r1, :], in_=xp[i, :, r0 * WP:r1 * WP]
                    )
            for i in range(2):
                nc.sync.dma_start(out=w2[i][:], in_=w2p[i])
            nc.sync.dma_start(out=pt[:], in_=ptp[:])
            nc.sync.dma_start(out=b1[:], in_=b1p[:])
            nc.sync.dma_start(out=b2[:], in_=b2p[:])
            nc.vector.memset(ones128[:], 1.0)
            nc.vector.memset(ones1[:], 1.0)
            # hpad border zeros only (interior is fully overwritten by conv1)
            for i in range(2):
                nc.gpsimd.memset(hpad[i][:, 0:1, :], 0.0)
                nc.gpsimd.memset(hpad[i][:, HP - 1:HP, :], 0.0)
                nc.gpsimd.memset(hpad[i][:, 1:HP - 1, 0:1], 0.0)
                nc.gpsimd.memset(hpad[i][:, 1:HP - 1, WP - 1:WP], 0.0)

            def conv1_block(y0, R):
                for cot in range(2):
                    ph = psA.tile([128, 5, 96], F32, tag=f"ph{cot}", name=f"ph{cot}")
                    k = 0
                    for tap in range(9):
                        ty, tx = divmod(tap, 3)
                        for cit in range(2):
                            nc.tensor.matmul(
                                out=ph[:, :R, :],
                                lhsT=w1[cit][:, (tap * 2 + cot) * 128:(tap * 2 + cot + 1) * 128],
                                rhs=xpad[cit][:, y0 + ty:y0 + ty + R, tx:tx + 96],
                                start=(k == 0),
                                stop=(k == 17),
                            )
                            k += 1
                    nc.vector.tensor_scalar_add(
                        out=hpad[cot][:, y0 + 1:y0 + 1 + R, 1:97],
                        in0=ph[:, :R, :],
                        scalar1=b1[:, cot:cot + 1],
                    )

            def conv2_block(y0, R):
                # normalized embedding into embn[:, y0*96 : (y0+R)*96]
                pe = psB.tile([128, 5, 96], F32, tag="pe", name="pe")
                k = 0
                for tap in range(9):
                    ty, tx = divmod(tap, 3)
                    for cit in range(2):
                        nc.tensor.matmul(
                            out=pe[:, :R, :],
                            lhsT=w2[cit][:, tap * 128:(tap + 1) * 128],
                            rhs=hpad[cit][:, y0 + ty:y0 + ty + R, tx:tx + 96],
                            start=(k == 0),
                            stop=(k == 17),
                        )
                        k += 1
                embb = wk.tile([128, 5, 96], F32, tag="embb", bufs=2, name="embb")
                nc.vector.tensor_scalar_add(
                    out=embb[:, :R, :], in0=pe[:, :R, :], scalar1=b2[:, 0:1]
                )
                sq = wk.tile([128, 5, 96], BF16, tag="sq", bufs=2, name="sq")
                nc.vector.tensor_mul(
                    out=sq[:, :R, :], in0=embb[:, :R, :], in1=embb[:, :R, :]
                )
                pss = psB.tile([1, 5, 96], F32, tag="pss", name="pss")
                nc.tensor.matmul(
                    out=pss[:, :R, :], lhsT=ones128[:, 0:1], rhs=sq[:, :R, :]
                )
                # rn = 1/sqrt(ss) = exp(-0.5*ln(ss)) — keeps all ACT work in the
                # natural_log_exp set (no table reload) and avoids the slow
                # DVE reciprocal entirely
                lnss = wk.tile([1, 5, 96], F32, tag="lnss", bufs=2, name="lnss")
                ia = nc.scalar.activation(out=lnss[:, :R, :], in_=pss[:, :R, :], func=AF.Ln)
                if act_order["last_sqrt"] is not None:
                    add_dep_helper(ia.ins, act_order["last_sqrt"], reason="act-phase")
                rnb = wk.tile([1, 5, 96], BF16, tag="rnb", bufs=2, name="rnb")
                nc.scalar.activation(
                    out=rnb[:, :R, :], in_=lnss[:, :R, :], func=AF.Exp, scale=-0.5
                )
                # broadcast rn across partitions via PE outer product
                prb = psB.tile([128, 5, 96], F32, tag="prb", name="prb")
                nc.tensor.matmul(
                    out=prb[:, :R, :], lhsT=ones1[0:1, :], rhs=rnb[0:1, :R, :]
                )
                nc.vector.tensor_mul(
                    out=embn[:, y0 * 96:(y0 + R) * 96],
                    in0=embb[:, :R, :],
                    in1=prb[:, :R, :],
                )

            def dots_super(s):
                c0 = s * SW  # pixel offset
                dist = wk.tile([NCLS, SW], F32, tag="dist", bufs=2, name="dist")
                dn = [
                    wk.tile([NCLS, SW], F32, tag=f"dn{m}", bufs=2, name=f"dn{m}")
                    for m in range(3)
                ]
                # ---- sqrt-table phase: dot matmuls + distance sqrt ----
                for half in range(2):
                    p0 = c0 + half * CW
                    for j in range(4):
                        pd = psD.tile([NCLS, CW], F32, tag="dot", bufs=3, name="pd")
                        nc.tensor.matmul(
                            out=pd[:, :],
                            lhsT=pt[:, j * NCLS:(j + 1) * NCLS],
                            rhs=embn[:, p0:p0 + CW],
                        )
                        dstt = dist if j == 0 else dn[j - 1]
                        isq = nc.scalar.activation(
                            out=dstt[:, half * CW:(half + 1) * CW], in_=pd[:, :],
                            func=AF.Sqrt, scale=-2.0, bias=2.0,
                        )
                        if half == 0 and j == 0 and act_order["last_lnexp"] is not None:
                            add_dep_helper(isq.ins, act_order["last_lnexp"], reason="act-phase")
                        act_order["last_sqrt"] = isq.ins
                nc.sync.dma_start(out=dst_o[:, c0:c0 + SW], in_=dist[:, :])
                for m in range(3):
                    nc.sync.dma_start(out=dsn_o[:, m, c0:c0 + SW], in_=dn[m][:, :])

                # ---- ln/exp/relu/square-table phase ----
                tmp = wk.tile([NCLS, SW], F32, tag="tmp", bufs=2, name="tmp")
                po = wk.tile([NCLS, SW], F32, tag="po", bufs=2, name="po")
                # probs_ori = exp(-2*dist^2)
                nc.scalar.activation(out=tmp[:, :], in_=dist[:, :], func=AF.Square)
                nc.scalar.activation(out=po[:, :], in_=tmp[:, :], func=AF.Exp, scale=-2.0)
                nc.sync.dma_start(out=po_o[:, c0:c0 + SW], in_=po[:, :])
                # min over m of distances_neg  (gpsimd: SBUF-only elementwise)
                mn = wk.tile([NCLS, SW], F32, tag="mn", bufs=2, name="mn")
                nc.vector.tensor_tensor(out=mn[:, :], in0=dn[0][:, :], in1=dn[1][:, :], op=ALU.min)
                nc.vector.tensor_tensor(out=mn[:, :], in0=mn[:, :], in1=dn[2][:, :], op=ALU.min)
                # cls_score_neg = exp(-2*mn^2)
                nc.scalar.activation(out=tmp[:, :], in_=mn[:, :], func=AF.Square)
                nc.scalar.activation(out=tmp[:, :], in_=tmp[:, :], func=AF.Exp, scale=-2.0)
                nc.sync.dma_start(out=csn_o[:, c0:c0 + SW], in_=tmp[:, :])
                # u = relu(0.6 - 0.3*mn); shifted = dist + u; s2 = shifted^2
                nc.scalar.activation(out=mn[:, :], in_=mn[:, :], func=AF.Relu, scale=-0.3, bias=0.6)
                nc.vector.tensor_add(out=dist[:, :], in0=dist[:, :], in1=mn[:, :])
                nc.scalar.activation(out=dist[:, :], in_=dist[:, :], func=AF.Square)
                p = wk.tile([NCLS, SW], F32, tag="p", bufs=2, name="p")
                nc.scalar.activation(out=p[:, :], in_=dist[:, :], func=AF.Exp, scale=-2.0)
                q = wk.tile([NCLS, SW], F32, tag="q", bufs=2, name="q")
                nc.vector.tensor_scalar(
                    out=q[:, :], in0=p[:, :], scalar1=-1.0, scalar2=1.0,
                    op0=ALU.mult, op1=ALU.add,
                )
                nc.vector.tensor_scalar_max(out=p[:, :], in0=p[:, :], scalar1=1e-5)
                nc.scalar.activation(out=p[:, :], in_=p[:, :], func=AF.Ln)
                nc.vector.tensor_scalar_max(out=q[:, :], in0=q[:, :], scalar1=1e-5)
                ilb = nc.scalar.activation(out=q[:, :], in_=q[:, :], func=AF.Ln)
                act_order["last_lnexp"] = ilb.ins
                nc.vector.tensor_sub(out=q[:, :], in0=p[:, :], in1=q[:, :])
                nc.sync.dma_start(out=cls_o[:, c0:c0 + SW], in_=q[:, :])

            # ---- interleaved schedule ----
            act_order = {"last_sqrt": None, "last_lnexp": None}
            n_c1 = 0
            n_b1 = 0

            def emit_b1_upto(target):
                nonlocal n_c1, n_b1
                while n_b1 < target:
                    while n_c1 < min(len(BLOCKS), n_b1 + 2):
                        conv1_block(*BLOCKS[n_c1])
                        n_c1 += 1
                    conv2_block(*BLOCKS[n_b1])
                    n_b1 += 1

            for s in range(NSUP):
                need_rows = min(H, (s + 1) * SW // W + 1)
                emit_b1_upto(min(len(BLOCKS), (need_rows + 4) // 5))
                dots_super(s)

    # this walrus build rejects instructions with >1 embedded sem wait;
    # fail fast here instead of minutes later inside neuronx-cc
    for name, inst in nc.inst_map.items():
        si = inst.sync_info
        nw = len(si.on_wait) if si and si.on_wait else 0
        assert nw <= 1, f"{name} ({type(inst).__name__}) has {nw} sem waits"
    return nc


def _l2norm_np(v, axis):
    n = np.maximum(np.linalg.norm(v, axis=axis, keepdims=True), 1e-12)
    return v / n


def _prep_host(x, conv1_w, conv1_b, bn_gamma, bn_beta, bn_mean, bn_var,
               conv2_w, conv2_b, rep_w, rep_b, neg_w, neg_b):
    """Host-side packing: BN fold, weight layout, prototype MLPs, input pad."""
    f32 = np.float32
    x = np.asarray(x, f32)
    conv1_w = np.asarray(conv1_w, f32)
    conv1_b = np.asarray(conv1_b, f32)
    scale = np.asarray(bn_gamma, f32) / np.sqrt(np.asarray(bn_var, f32) + 1e-5)
    w1f = conv1_w * scale[:, None, None, None]
    b1f = (conv1_b - np.asarray(bn_mean, f32)) * scale + np.asarray(bn_beta, f32)

    # conv1 weights -> [cit, ci, (ky kx cot co)]
    w1r = w1f.reshape(2, 128, 2, 128, 3, 3)  # [cot, co, cit, ci, ky, kx]
    w1p = np.ascontiguousarray(
        w1r.transpose(2, 3, 4, 5, 0, 1).reshape(2, 128, 9 * 2 * 128)
    ).astype(ml_dtypes.bfloat16)
    # conv2 weights -> [cit, ci, (ky kx co)]
    w2r = np.asarray(conv2_w, f32).reshape(128, 2, 128, 3, 3)  # [co, cit, ci, ky, kx]
    w2p = np.ascontiguousarray(
        w2r.transpose(1, 2, 3, 4, 0).reshape(2, 128, 9 * 128)
    ).astype(ml_dtypes.bfloat16)

    b1p = np.ascontiguousarray(b1f.reshape(2, 128).T)  # [co_within, cot]
    b2p = np.asarray(conv2_b, f32).reshape(128, 1).copy()

    # prototypes (tiny) on host, fp32
    reps = (np.asarray(rep_w, f32)[:, 0] + np.asarray(rep_b, f32)).reshape(NCLS, EMB2)
    r = _l2norm_np(reps, 1)  # [C, E]
    nw = np.asarray(neg_w, f32)
    nb = np.asarray(neg_b, f32)
    rn = np.empty((NEG, NCLS, EMB2), f32)
    for m in range(NEG):
        h = r
        for i in range(NLAY):
            h = h @ nw[m, i].T + nb[m, i]
            if i < NLAY - 1:
                h = np.maximum(h, 0.0)
        rn[m] = h
    rn = _l2norm_np(np.transpose(rn, (1, 0, 2)), 2)  # [C, NEG, E]

    ptp = np.zeros((128, 4 * NCLS), f32)
    ptp[:, :NCLS] = r.T
    for m in range(NEG):
        ptp[:, NCLS + m * NCLS:NCLS + (m + 1) * NCLS] = rn[:, m, :].T
    ptp = ptp.astype(ml_dtypes.bfloat16)

    # per-image padded bf16 input [2, 128, HP*WP]
    xpads = []
    for b in range(B):
        xi = np.zeros((2, 128, HP, WP), f32)
        xi[:, :, 1:97, 1:97] = x[b].reshape(2, 128, H, W)
        xpads.append(xi.reshape(2, 128, HP * WP).astype(ml_dtypes.bfloat16))

    shared = {"w1p": w1p, "w2p": w2p, "ptp": ptp, "b1p": b1p, "b2p": b2p}
    return xpads, shared


_NC_CACHE = {}


def kernel(**inputs) -> tuple:
    from concourse.bass_utils import run_bass_kernel_spmd

    xpads, shared = _prep_host(**inputs)
    if "nc" not in _NC_CACHE:
        _NC_CACHE["nc"] = build_nc()
    nc = _NC_CACHE["nc"]

    in_maps = [dict(shared, xp=xpads[b]) for b in range(B)]
    res = run_bass_kernel_spmd(nc, in_maps, list(range(N_CORES)))

    f32 = np.float32
    cls = np.stack([np.asarray(res.results[b]["cls_o"], f32).reshape(NCLS, H, W) for b in range(B)])
    csn = np.stack([np.asarray(res.results[b]["csn_o"], f32).reshape(NCLS, H, W) for b in range(B)])
    dst = np.stack([np.asarray(res.results[b]["dst_o"], f32).reshape(NCLS, 1, H, W) for b in range(B)])
    dsn = np.stack([np.asarray(res.results[b]["dsn_o"], f32).reshape(NCLS, NEG, H, W) for b in range(B)])
    po = np.stack([np.asarray(res.results[b]["po_o"], f32).reshape(NCLS, H, W) for b in range(B)])
    return (cls, csn, dst, dsn, po)
